# revision 2
# baseline (speedup 1.0000x reference)
"""ConvAttention (XCA-style channel attention) Trainium2 Bass kernel.

Reference computation (per batch element n, DIM=192, HEADS=6, H=W=128):
    qkv = conv3x3(x, qkv_w)                  # [576, H, W]
    q, k, v = split(qkv)                     # each [192, H*W]
    q = q / max(||q||_hw, eps); k likewise   # L2 norm over spatial
    attn = softmax(q @ k^T * temp, axis=-1)  # per-head [32, 32]
    out = attn @ v                           # [192, H*W]
    out = proj_w @ out + proj_b

Strategy (8 cores, data-parallel over batch N=8 -> 1 image per core):
  - 3x3 conv done as 9 shifted matmuls over a zero-padded [192,130,130]
    bf16 image kept in SBUF row-windows.
  - q,k produced SPATIAL-major ([128 positions, 384 ch] psum tiles) so the
    Gram matrix J^T J (J=[q|k]) accumulates directly on PE with the
    spatial dim as contraction. Norms = diag of the Gram.
  - v produced CHANNEL-major ([192 ch, 512 positions]) and kept in SBUF.
  - softmax of the 6 32x32 blocks built from the Gram on DVE/ACT; the
    projection is folded in: M^T = BlockDiag(A)^T @ proj_w^T, and the
    final output is one [192,192] @ [192,16384] matmul + bias.
"""

import numpy as np
import ml_dtypes

import concourse.bass as bass
import concourse.tile as tile
from concourse import bacc, mybir
from concourse.bass_utils import run_bass_kernel_spmd

F32 = mybir.dt.float32
BF16 = mybir.dt.bfloat16
AF = mybir.ActivationFunctionType
ALU = mybir.AluOpType
AX = mybir.AxisListType

DIM = 192
HEADS = 6
H = W = 128
HP = WP = 130  # padded
NPOS = H * W
NCORES = 8
EPS = 1e-12

_CACHE = {}


def _emit(tc):
    nc = tc.nc
    xp = nc.dram_tensor("xp", [DIM, HP, WP], BF16, kind="ExternalInput").ap()
    wqk = nc.dram_tensor("wqk", [DIM, 9, 384], BF16, kind="ExternalInput").ap()
    wv = nc.dram_tensor("wv", [DIM, 9, 192], BF16, kind="ExternalInput").ap()
    wpt = nc.dram_tensor("wpt", [DIM, 192], BF16, kind="ExternalInput").ap()
    bias = nc.dram_tensor("bias", [DIM, 1], F32, kind="ExternalInput").ap()
    tempq = nc.dram_tensor("tempq", [DIM, 1], F32, kind="ExternalInput").ap()
    maskblk = nc.dram_tensor("maskblk", [DIM, 192], F32, kind="ExternalInput").ap()
    ident = nc.dram_tensor("ident", [128, 128], F32, kind="ExternalInput").ap()
    out = nc.dram_tensor("out", [DIM, NPOS], F32, kind="ExternalOutput").ap()
    tkd = nc.dram_tensor("tkd", [1, 192], F32).ap()  # internal bounce

    import contextlib

    with contextlib.ExitStack() as ctx:
        # ---------- persistent SBUF ----------
        pers = ctx.enter_context(tc.tile_pool(name="pers", bufs=1))
        wqk_lo = pers.tile([128, 9, 384], BF16)
        wqk_hi = pers.tile([64, 9, 384], BF16)
        wv_lo = pers.tile([128, 9, 192], BF16)
        wv_hi = pers.tile([64, 9, 192], BF16)
        wpt_lo = pers.tile([128, 192], BF16)
        wpt_hi = pers.tile([64, 192], BF16)
        bias_lo = pers.tile([128, 1], F32)
        bias_hi = pers.tile([64, 1], F32)
        tempq_lo = pers.tile([128, 1], F32)
        tempq_hi = pers.tile([64, 1], F32)
        mask_lo = pers.tile([128, 192], F32)
        mask_hi = pers.tile([64, 192], F32)
        ident_sb = pers.tile([128, 128], F32)
        v_lo = pers.tile([128, NPOS], BF16)
        v_hi = pers.tile([64, NPOS], BF16)
        mt0 = pers.tile([128, 192], BF16)
        mt1 = pers.tile([64, 192], BF16)

        nc.sync.dma_start(wqk_lo[:], wqk[0:128])
        nc.sync.dma_start(wqk_hi[:], wqk[128:192])
        nc.sync.dma_start(wv_lo[:], wv[0:128])
        nc.sync.dma_start(wv_hi[:], wv[128:192])
        nc.sync.dma_start(wpt_lo[:], wpt[0:128])
        nc.sync.dma_start(wpt_hi[:], wpt[128:192])
        nc.sync.dma_start(bias_lo[:], bias[0:128])
        nc.sync.dma_start(bias_hi[:], bias[128:192])
        nc.sync.dma_start(tempq_lo[:], tempq[0:128])
        nc.sync.dma_start(tempq_hi[:], tempq[128:192])
        nc.sync.dma_start(mask_lo[:], maskblk[0:128])
        nc.sync.dma_start(mask_hi[:], maskblk[128:192])
        nc.sync.dma_start(ident_sb[:], ident[:])

        # ---------- Gram accumulator (PSUM, lives through phase A+B) ----------
        gram_pool = ctx.enter_context(
            tc.tile_pool(name="gram", bufs=1, space="PSUM")
        )
        gram0 = gram_pool.tile([128, 384], F32)
        gram1 = gram_pool.tile([128, 384], F32)
        gram2 = gram_pool.tile([128, 384], F32)
        gram = [gram0, gram1, gram2]

        # ================= phase A: conv + gram + v =================
        with tc.tile_pool(name="xbp", bufs=2) as xbp, \
             tc.tile_pool(name="jp", bufs=3) as jp, \
             tc.tile_pool(name="pqkp", bufs=2, space="PSUM") as pqkp, \
             tc.tile_pool(name="pvp", bufs=1, space="PSUM") as pvp:
            for g in range(32):  # 4 output rows per group
                xb_lo = xbp.tile([128, 6, WP], BF16, tag="xlo")
                xb_hi = xbp.tile([64, 6, WP], BF16, tag="xhi")
                nc.sync.dma_start(xb_lo[:], xp[0:128, 4 * g : 4 * g + 6, :])
                nc.sync.dma_start(xb_hi[:], xp[128:192, 4 * g : 4 * g + 6, :])

                # ---- v: channel-major conv, N=512 (4 rows) ----
                pv0 = pvp.tile([128, 4, 128], F32, tag="pv0")
                pv1 = pvp.tile([64, 4, 128], F32, tag="pv1")
                for t in range(9):
                    dy, dx = divmod(t, 3)
                    rl = xb_lo[:, dy : dy + 4, dx : dx + 128]
                    rh = xb_hi[:, dy : dy + 4, dx : dx + 128]
                    nc.tensor.matmul(pv0[:], lhsT=wv_lo[:, t, 0:128], rhs=rl,
                                     start=(t == 0), stop=False)
                    nc.tensor.matmul(pv0[:], lhsT=wv_hi[:, t, 0:128], rhs=rh,
                                     start=False, stop=(t == 8))
                    nc.tensor.matmul(pv1[:], lhsT=wv_lo[:, t, 128:192], rhs=rl,
                                     start=(t == 0), stop=False)
                    nc.tensor.matmul(pv1[:], lhsT=wv_hi[:, t, 128:192], rhs=rh,
                                     start=False, stop=(t == 8))
                nc.vector.tensor_copy(
                    v_lo[:, 512 * g : 512 * (g + 1)],
                    pv0[:].rearrange("p a b -> p (a b)"),
                )
                nc.vector.tensor_copy(
                    v_hi[:, 512 * g : 512 * (g + 1)],
                    pv1[:].rearrange("p a b -> p (a b)"),
                )

                # ---- q,k: spatial-major conv + gram accumulation ----
                for r in range(4):
                    y = 4 * g + r
                    pqk = pqkp.tile([128, 384], F32, tag="pqk")
                    for t in range(9):
                        dy, dx = divmod(t, 3)
                        nc.tensor.matmul(
                            pqk[:], lhsT=xb_lo[:, r + dy, dx : dx + 128],
                            rhs=wqk_lo[:, t, :], start=(t == 0), stop=False)
                        nc.tensor.matmul(
                            pqk[:], lhsT=xb_hi[:, r + dy, dx : dx + 128],
                            rhs=wqk_hi[:, t, :], start=False, stop=(t == 8))
                    jt = jp.tile([128, 384], BF16, tag="jt")
                    nc.vector.tensor_copy(jt[:], pqk[:])
                    for m in range(3):
                        nc.tensor.matmul(
                            gram[m][:], lhsT=jt[:, 128 * m : 128 * (m + 1)],
                            rhs=jt[:], start=(y == 0), stop=(y == 127))

        # ================= phase B: norms, softmax, fold projection ==========
        with tc.tile_pool(name="phb", bufs=1) as phb, \
             tc.tile_pool(name="pmtp", bufs=1, space="PSUM") as pmtp:
            R = [phb.tile([128, 384], F32, tag=f"R{m}", name=f"R{m}")
                 for m in range(3)]
            for m in range(3):
                nc.vector.tensor_copy(R[m][:], gram[m][:])

            inv = []
            for m in range(3):
                dtmp = phb.tile([128, 128], F32, tag=f"dtmp{m}")
                nc.vector.tensor_mul(
                    dtmp[:], R[m][:, 128 * m : 128 * (m + 1)], ident_sb[:])
                sq = phb.tile([128, 1], F32, tag=f"sq{m}")
                nc.vector.tensor_reduce(sq[:], dtmp[:], axis=AX.X, op=ALU.add)
                nrm = phb.tile([128, 1], F32, tag=f"nrm{m}")
                nc.scalar.sqrt(nrm[:], sq[:])
                nc.vector.tensor_scalar_max(nrm[:], nrm[:], EPS)
                iv = phb.tile([128, 1], F32, tag=f"iv{m}")
                nc.vector.reciprocal(iv[:], nrm[:])
                inv.append(iv)

            # k-channel inverse norms -> one [1,192] row via DRAM bounce,
            # then broadcast to 128 partitions.
            nc.sync.dma_start(tkd[0:1, 0:64], inv[1][64:128, :])
            nc.sync.dma_start(tkd[0:1, 64:192], inv[2][:])
            colfac = phb.tile([128, 192], F32)
            nc.sync.dma_start(colfac[:], tkd[0:1, :].to_broadcast((128, 192)))

            rowA = phb.tile([128, 1], F32)
            nc.vector.tensor_mul(rowA[:], inv[0][:], tempq_lo[:])
            rowB = phb.tile([64, 1], F32)
            nc.vector.tensor_mul(rowB[:], inv[1][0:64, :], tempq_hi[:])

            a_chunks = []
            for name, rows, gsl, rowfac, msk in (
                ("A0", 128, R[0][:, 192:384], rowA, mask_lo),
                ("A1", 64, R[1][0:64, 192:384], rowB, mask_hi),
            ):
                tl = phb.tile([rows, 192], F32, tag=f"tl{name}")
                nc.vector.tensor_mul(tl[:], gsl, colfac[0:rows, :])
                ex = phb.tile([rows, 192], F32, tag=f"ex{name}")
                nc.scalar.activation(ex[:], tl[:], AF.Exp, scale=rowfac[:])
                nc.vector.tensor_mul(ex[:], ex[:], msk[:])
                den = phb.tile([rows, 1], F32, tag=f"den{name}")
                nc.vector.tensor_reduce(den[:], ex[:], axis=AX.X, op=ALU.add)
                rden = phb.tile([rows, 1], F32, tag=f"rden{name}")
                nc.vector.reciprocal(rden[:], den[:])
                ab = phb.tile([rows, 192], BF16, tag=f"ab{name}")
                nc.vector.tensor_scalar_mul(ab[:], ex[:], rden[:])
                a_chunks.append(ab)
            a0, a1 = a_chunks

            # M^T = BlockDiag(A)^T @ proj_w^T   -> [192(d), 192(o)]
            pmt0 = pmtp.tile([128, 192], F32, tag="pmt0")
            pmt1 = pmtp.tile([64, 192], F32, tag="pmt1")
            nc.tensor.matmul(pmt0[:], lhsT=a0[:, 0:128], rhs=wpt_lo[:],
                             start=True, stop=False)
            nc.tensor.matmul(pmt0[:], lhsT=a1[:, 0:128], rhs=wpt_hi[:],
                             start=False, stop=True)
            nc.tensor.matmul(pmt1[:], lhsT=a0[:, 128:192], rhs=wpt_lo[:],
                             start=True, stop=False)
            nc.tensor.matmul(pmt1[:], lhsT=a1[:, 128:192], rhs=wpt_hi[:],
                             start=False, stop=True)
            nc.vector.tensor_copy(mt0[:], pmt0[:])
            nc.vector.tensor_copy(mt1[:], pmt1[:])

        # ================= phase C: out = M^T.T @ v + bias =================
        with tc.tile_pool(name="obp", bufs=3) as obp, \
             tc.tile_pool(name="pfp", bufs=2, space="PSUM") as pfp:
            for s in range(32):
                sl = slice(512 * s, 512 * (s + 1))
                pf0 = pfp.tile([128, 512], F32, tag="pf0")
                pf1 = pfp.tile([64, 512], F32, tag="pf1")
                nc.tensor.matmul(pf0[:], lhsT=mt0[:, 0:128], rhs=v_lo[:, sl],
                                 start=True, stop=False)
                nc.tensor.matmul(pf0[:], lhsT=mt1[:, 0:128], rhs=v_hi[:, sl],
                                 start=False, stop=True)
                nc.tensor.matmul(pf1[:], lhsT=mt0[:, 128:192], rhs=v_lo[:, sl],
                                 start=True, stop=False)
                nc.tensor.matmul(pf1[:], lhsT=mt1[:, 128:192], rhs=v_hi[:, sl],
                                 start=False, stop=True)
                ob0 = obp.tile([128, 512], F32, tag="ob0")
                ob1 = obp.tile([64, 512], F32, tag="ob1")
                nc.vector.tensor_scalar_add(ob0[:], pf0[:], bias_lo[:])
                nc.vector.tensor_scalar_add(ob1[:], pf1[:], bias_hi[:])
                nc.sync.dma_start(out[0:128, sl], ob0[:])
                nc.sync.dma_start(out[128:192, sl], ob1[:])


def build_program():
    if "nc" in _CACHE:
        return _CACHE["nc"]
    nc = bacc.Bacc("TRN2", target_bir_lowering=False, debug=False,
                   num_devices=NCORES)
    with tile.TileContext(nc) as tc:
        _emit(tc)
    nc.compile()
    _CACHE["nc"] = nc
    return nc


def prep_in_maps(x, qkv_w, proj_w, proj_b, temperature):
    bf16 = ml_dtypes.bfloat16
    n = x.shape[0]
    assert x.shape == (n, DIM, H, W)
    xpad = np.zeros((n, DIM, HP, WP), dtype=bf16)
    xpad[:, :, 1 : H + 1, 1 : W + 1] = x.astype(bf16)
    wqk = np.ascontiguousarray(
        qkv_w[: 2 * DIM].transpose(1, 2, 3, 0).reshape(DIM, 9, 2 * DIM)
    ).astype(bf16)
    wv = np.ascontiguousarray(
        qkv_w[2 * DIM :].transpose(1, 2, 3, 0).reshape(DIM, 9, DIM)
    ).astype(bf16)
    wpt = np.ascontiguousarray(proj_w[:, :, 0, 0].T).astype(bf16)
    biasc = np.ascontiguousarray(proj_b.reshape(DIM, 1)).astype(np.float32)
    tq = np.repeat(np.asarray(temperature, np.float32).reshape(HEADS), DIM // HEADS)
    tq = np.ascontiguousarray(tq.reshape(DIM, 1))
    mask = np.kron(np.eye(HEADS, dtype=np.float32),
                   np.ones((DIM // HEADS, DIM // HEADS), np.float32))
    mask = np.ascontiguousarray(mask)
    idn = np.eye(128, dtype=np.float32)
    shared = {"wqk": wqk, "wv": wv, "wpt": wpt, "bias": biasc,
              "tempq": tq, "maskblk": mask, "ident": idn}
    return [{"xp": np.ascontiguousarray(xpad[i]), **shared} for i in range(n)]


def kernel(x, qkv_w, proj_w, proj_b, temperature):
    x = np.asarray(x, np.float32)
    qkv_w = np.asarray(qkv_w, np.float32)
    proj_w = np.asarray(proj_w, np.float32)
    proj_b = np.asarray(proj_b, np.float32)
    temperature = np.asarray(temperature, np.float32)
    nc = build_program()
    in_maps = prep_in_maps(x, qkv_w, proj_w, proj_b, temperature)
    res = run_bass_kernel_spmd(nc, in_maps, core_ids=list(range(NCORES)))
    outs = [res.results[i]["out"].reshape(DIM, H, W) for i in range(NCORES)]
    return np.stack(outs, axis=0).astype(np.float32)


# revision 8
# speedup vs baseline: 1.0419x; 1.0419x over previous
"""ConvAttention (XCA-style channel attention) Trainium2 Bass kernel.

Reference computation (per batch element n, DIM=192, HEADS=6, H=W=128):
    qkv = conv3x3(x, qkv_w)                  # [576, H, W]
    q, k, v = split(qkv)                     # each [192, H*W]
    q = q / max(||q||_hw, eps); k likewise   # L2 norm over spatial
    attn = softmax(q @ k^T * temp, axis=-1)  # per-head [32, 32]
    out = attn @ v                           # [192, H*W]
    out = proj_w @ out + proj_b

Strategy (8 cores, data-parallel over batch N=8 -> 1 image per core):
  - 3x3 conv done as shifted matmuls over a zero-padded flat bf16 image.
    The contraction axis (192 ch x 9 taps = 1728) is re-chunked into 14
    dense K=128 chunks: 9 chunks = channels 0-127 of each tap; 5 chunks
    pack channels 128-191 of two adjacent taps by keeping an extra SBUF
    copy of the high channels shifted by the inter-tap offset (+1/+128).
  - Phase A computes only q,k, SPATIAL-major ([128 positions, 384 ch]
    psum tiles), so the Gram matrix J^T J (J=[q|k]) accumulates directly
    on PE with the spatial dim as contraction. Norms = Gram diagonal.
  - Phase B builds the 6 32x32 softmax blocks A from the Gram on
    DVE/ACT, then folds attention AND projection into effective conv
    weights: FW_t = proj_w @ BlockDiag(A) @ Wv_t  (on PE, tiny).
  - Phase C: out = conv3x3(x, FW) + bias — same dense-chunk conv as
    phase A, channel-major output. v is never materialized.
"""

import numpy as np
import ml_dtypes

import concourse.bass as bass
import concourse.tile as tile
from concourse import bacc, mybir
from concourse.bass_utils import run_bass_kernel_spmd

F32 = mybir.dt.float32
BF16 = mybir.dt.bfloat16
AF = mybir.ActivationFunctionType
ALU = mybir.AluOpType
AX = mybir.AxisListType

DIM = 192
HEADS = 6
H = W = 128
HP = WP = 130  # padded
XF = 17028  # flat padded length + 128 slop for the +128-shifted window
NPOS = H * W
NCORES = 8
EPS = 1e-12

# contraction chunks: (source, tap) where source 0=lo, 1=hi shifted +1,
# 2=hi shifted +128. Chunk 13 pairs hi@t8 with zero weights.
CHUNKS = [(0, t) for t in range(9)] + [(1, 0), (2, 2), (1, 4), (1, 6), (1, 8)]

_CACHE = {}


def _emit(tc):
    nc = tc.nc
    xp = nc.dram_tensor("xp", [DIM, XF], BF16, kind="ExternalInput").ap()
    wqk = nc.dram_tensor("wqk", [128, 14, 384], BF16, kind="ExternalInput").ap()
    wvs = nc.dram_tensor("wvs", [DIM, 14, 128], BF16, kind="ExternalInput").ap()
    wpt = nc.dram_tensor("wpt", [DIM, 192], BF16, kind="ExternalInput").ap()
    bias = nc.dram_tensor("bias", [DIM, 1], F32, kind="ExternalInput").ap()
    tempq = nc.dram_tensor("tempq", [DIM, 1], F32, kind="ExternalInput").ap()
    maskblk = nc.dram_tensor("maskblk", [DIM, 192], F32, kind="ExternalInput").ap()
    ident = nc.dram_tensor("ident", [128, 128], F32, kind="ExternalInput").ap()
    out = nc.dram_tensor("out", [DIM, NPOS], F32, kind="ExternalOutput").ap()
    tkd = nc.dram_tensor("tkd", [1, 192], F32).ap()  # internal bounce

    import contextlib

    def _win(sl, b, off):
        return xp[sl, b + off : b + off + 780].rearrange(
            "p (a c) -> p a c", a=6, c=WP)

    def _load_windows(pool, g):
        b = 520 * g
        xb_lo = pool.tile([128, 6, WP], BF16, tag="xlo", name="xb_lo")
        xb_h1 = pool.tile([128, 6, WP], BF16, tag="xh1", name="xb_h1")
        xb_h2 = pool.tile([128, 6, WP], BF16, tag="xh2", name="xb_h2")
        nc.sync.dma_start(xb_lo[:], _win(slice(0, 128), b, 0))
        nc.sync.dma_start(xb_h1[0:64], _win(slice(128, 192), b, 0))
        nc.sync.dma_start(xb_h1[64:128], _win(slice(128, 192), b, 1))
        nc.sync.dma_start(xb_h2[0:64], _win(slice(128, 192), b, 0))
        nc.sync.dma_start(xb_h2[64:128], _win(slice(128, 192), b, 128))
        return (xb_lo, xb_h1, xb_h2)

    with contextlib.ExitStack() as ctx:
        # ---------- persistent SBUF ----------
        pers = ctx.enter_context(tc.tile_pool(name="pers", bufs=1))
        wqk_sb = pers.tile([128, 14, 384], BF16)
        wvs_lo = pers.tile([128, 14, 128], BF16)
        wvs_hi = pers.tile([64, 14, 128], BF16)
        wpt_lo = pers.tile([128, 192], BF16)
        wpt_hi = pers.tile([64, 192], BF16)
        bias_lo = pers.tile([128, 1], F32)
        bias_hi = pers.tile([64, 1], F32)
        tempq_lo = pers.tile([128, 1], F32)
        tempq_hi = pers.tile([64, 1], F32)
        mask_lo = pers.tile([128, 192], F32)
        mask_hi = pers.tile([64, 192], F32)
        ident_sb = pers.tile([128, 128], F32)
        fw_sb = pers.tile([128, 14, 192], BF16)

        nc.sync.dma_start(wqk_sb[:], wqk[:])
        nc.sync.dma_start(wvs_lo[:], wvs[0:128])
        nc.sync.dma_start(wvs_hi[:], wvs[128:192])
        nc.sync.dma_start(wpt_lo[:], wpt[0:128])
        nc.sync.dma_start(wpt_hi[:], wpt[128:192])
        nc.sync.dma_start(bias_lo[:], bias[0:128])
        nc.sync.dma_start(bias_hi[:], bias[128:192])
        nc.sync.dma_start(tempq_lo[:], tempq[0:128])
        nc.sync.dma_start(tempq_hi[:], tempq[128:192])
        nc.sync.dma_start(mask_lo[:], maskblk[0:128])
        nc.sync.dma_start(mask_hi[:], maskblk[128:192])
        nc.sync.dma_start(ident_sb[:], ident[:])

        # ---------- Gram accumulator (PSUM, lives through phase A+B) ----------
        gram_pool = ctx.enter_context(
            tc.tile_pool(name="gram", bufs=1, space="PSUM")
        )
        gram0 = gram_pool.tile([128, 384], F32)
        gram1 = gram_pool.tile([128, 256], F32)
        gram2 = gram_pool.tile([128, 128], F32)

        # ================= phase A: q,k conv + gram =================
        with tc.tile_pool(name="xbp", bufs=2) as xbp, \
             tc.tile_pool(name="jp", bufs=3) as jp, \
             tc.tile_pool(name="pqkp", bufs=3, space="PSUM") as pqkp:
            for g in range(32):  # 4 output rows per group
                xsrc = _load_windows(xbp, g)
                for r in range(4):
                    y = 4 * g + r
                    pqk = pqkp.tile([128, 384], F32, tag="pqk")
                    for j, (srci, t) in enumerate(CHUNKS):
                        dy, dx = divmod(t, 3)
                        nc.tensor.matmul(
                            pqk[:], lhsT=xsrc[srci][:, r + dy, dx : dx + 128],
                            rhs=wqk_sb[:, j, :], start=(j == 0), stop=(j == 13))
                    jt = jp.tile([128, 384], BF16, tag="jt")
                    nc.vector.tensor_copy(jt[:], pqk[:])
                    st, sp = (y == 0), (y == 127)
                    nc.tensor.matmul(gram0[:], lhsT=jt[:, 0:128], rhs=jt[:],
                                     start=st, stop=sp)
                    nc.tensor.matmul(gram1[:], lhsT=jt[:, 128:256],
                                     rhs=jt[:, 128:384], start=st, stop=sp)
                    nc.tensor.matmul(gram2[:], lhsT=jt[:, 256:384],
                                     rhs=jt[:, 256:384], start=st, stop=sp)

        # ========== phase B: norms, softmax, fold attn+proj into conv =======
        with tc.tile_pool(name="phb", bufs=1) as phb, \
             tc.tile_pool(name="pmtp", bufs=1, space="PSUM") as pmtp:
            R0 = phb.tile([128, 384], F32)
            R1 = phb.tile([128, 256], F32)
            R2 = phb.tile([128, 128], F32)
            nc.vector.tensor_copy(R0[:], gram0[:])
            nc.vector.tensor_copy(R1[:], gram1[:])
            nc.vector.tensor_copy(R2[:], gram2[:])

            inv = []
            for m, Rm in enumerate((R0, R1, R2)):
                dtmp = phb.tile([128, 128], F32, tag=f"dtmp{m}",
                                name=f"dtmp{m}")
                nc.vector.tensor_mul(dtmp[:], Rm[:, 0:128], ident_sb[:])
                sq = phb.tile([128, 1], F32, tag=f"sq{m}", name=f"sq{m}")
                nc.vector.tensor_reduce(sq[:], dtmp[:], axis=AX.X, op=ALU.add)
                nrm = phb.tile([128, 1], F32, tag=f"nrm{m}", name=f"nrm{m}")
                nc.scalar.sqrt(nrm[:], sq[:])
                nc.vector.tensor_scalar_max(nrm[:], nrm[:], EPS)
                iv = phb.tile([128, 1], F32, tag=f"iv{m}", name=f"iv{m}")
                nc.vector.reciprocal(iv[:], nrm[:])
                inv.append(iv)

            # k-channel inverse norms -> one [1,192] row via DRAM bounce,
            # then broadcast to 128 partitions.
            nc.sync.dma_start(tkd[0:1, 0:64], inv[1][64:128, :])
            nc.sync.dma_start(tkd[0:1, 64:192], inv[2][:])
            colfac = phb.tile([128, 192], F32)
            nc.sync.dma_start(colfac[:], tkd[0:1, :].to_broadcast((128, 192)))

            rowA = phb.tile([128, 1], F32)
            nc.vector.tensor_mul(rowA[:], inv[0][:], tempq_lo[:])
            rowB = phb.tile([64, 1], F32)
            nc.vector.tensor_mul(rowB[:], inv[1][0:64, :], tempq_hi[:])

            a_chunks = []
            for name, rows, gsl, rowfac, msk in (
                ("A0", 128, R0[:, 192:384], rowA, mask_lo),
                ("A1", 64, R1[0:64, 64:256], rowB, mask_hi),
            ):
                tl = phb.tile([rows, 192], F32, tag=f"tl{name}",
                              name=f"tl{name}")
                nc.vector.tensor_mul(tl[:], gsl, colfac[0:rows, :])
                ex = phb.tile([rows, 192], F32, tag=f"ex{name}",
                              name=f"ex{name}")
                nc.scalar.activation(ex[:], tl[:], AF.Exp, scale=rowfac[:])
                nc.vector.tensor_mul(ex[:], ex[:], msk[:])
                den = phb.tile([rows, 1], F32, tag=f"den{name}",
                               name=f"den{name}")
                nc.vector.tensor_reduce(den[:], ex[:], axis=AX.X, op=ALU.add)
                rden = phb.tile([rows, 1], F32, tag=f"rden{name}",
                                name=f"rden{name}")
                nc.vector.reciprocal(rden[:], den[:])
                ab = phb.tile([rows, 192], BF16, tag=f"ab{name}",
                              name=f"ab{name}")
                nc.vector.tensor_scalar_mul(ab[:], ex[:], rden[:])
                a_chunks.append(ab)
            a0, a1 = a_chunks

            # M^T = BlockDiag(A)^T @ proj_w^T   -> [192(d), 192(o)]
            pmt0 = pmtp.tile([128, 192], F32, tag="pmt0")
            pmt1 = pmtp.tile([64, 192], F32, tag="pmt1")
            nc.tensor.matmul(pmt0[:], lhsT=a0[:, 0:128], rhs=wpt_lo[:],
                             start=True, stop=False)
            nc.tensor.matmul(pmt0[:], lhsT=a1[:, 0:128], rhs=wpt_hi[:],
                             start=False, stop=True)
            nc.tensor.matmul(pmt1[:], lhsT=a0[:, 128:192], rhs=wpt_lo[:],
                             start=True, stop=False)
            nc.tensor.matmul(pmt1[:], lhsT=a1[:, 128:192], rhs=wpt_hi[:],
                             start=False, stop=True)
            mt0 = phb.tile([128, 192], BF16)
            mt1 = phb.tile([64, 192], BF16)
            nc.vector.tensor_copy(mt0[:], pmt0[:])
            nc.vector.tensor_copy(mt1[:], pmt1[:])

            # FW^T[(c,t), o] = sum_d Wv_stack[d,(c,t)] * M^T[d, o]
            for j in range(14):
                pfw = pmtp.tile([128, 192], F32, tag="pfw", name="pfw", bufs=2)
                nc.tensor.matmul(pfw[:], lhsT=wvs_lo[:, j, :], rhs=mt0[:],
                                 start=True, stop=False)
                nc.tensor.matmul(pfw[:], lhsT=wvs_hi[:, j, :], rhs=mt1[:],
                                 start=False, stop=True)
                nc.vector.tensor_copy(fw_sb[:, j, :], pfw[:])

        # ========= phase C: out = conv3x3(x, FW) + bias (channel-major) ======
        with tc.tile_pool(name="xcp", bufs=2) as xcp, \
             tc.tile_pool(name="obp", bufs=3) as obp, \
             tc.tile_pool(name="pfp", bufs=2, space="PSUM") as pfp:
            for g in range(32):
                xsrc = _load_windows(xcp, g)
                pf0 = pfp.tile([128, 4, 128], F32, tag="pf0")
                pf1 = pfp.tile([64, 4, 128], F32, tag="pf1")
                for j, (srci, t) in enumerate(CHUNKS):
                    dy, dx = divmod(t, 3)
                    rv = xsrc[srci][:, dy : dy + 4, dx : dx + 128]
                    nc.tensor.matmul(pf0[:], lhsT=fw_sb[:, j, 0:128], rhs=rv,
                                     start=(j == 0), stop=(j == 13))
                    nc.tensor.matmul(pf1[:], lhsT=fw_sb[:, j, 128:192], rhs=rv,
                                     start=(j == 0), stop=(j == 13))
                sl = slice(512 * g, 512 * (g + 1))
                ob0 = obp.tile([128, 512], F32, tag="ob0")
                ob1 = obp.tile([64, 512], F32, tag="ob1")
                nc.vector.tensor_scalar_add(
                    ob0[:], pf0[:].rearrange("p a c -> p (a c)"), bias_lo[:])
                nc.vector.tensor_scalar_add(
                    ob1[:], pf1[:].rearrange("p a c -> p (a c)"), bias_hi[:])
                nc.sync.dma_start(out[0:128, sl], ob0[:])
                nc.sync.dma_start(out[128:192, sl], ob1[:])


def build_program():
    if "nc" in _CACHE:
        return _CACHE["nc"]
    nc = bacc.Bacc("TRN2", target_bir_lowering=False, debug=False,
                   num_devices=NCORES)
    with tile.TileContext(nc) as tc:
        _emit(tc)
    nc.compile()
    _CACHE["nc"] = nc
    return nc


def _pack_qk_weights(w):
    """w: [384, 192, 3, 3] fp32 -> [128, 14, 384] bf16 chunk layout."""
    nout = w.shape[0]
    wt = np.ascontiguousarray(w.transpose(1, 2, 3, 0).reshape(DIM, 9, nout))
    packed = np.zeros((128, 14, nout), dtype=np.float32)
    for j in range(9):
        packed[:, j, :] = wt[0:128, j, :]
    for j, t in enumerate((0, 2, 4, 6, 8)):
        packed[0:64, 9 + j, :] = wt[128:192, t, :]
        if t + 1 < 9:
            packed[64:128, 9 + j, :] = wt[128:192, t + 1, :]
    return np.ascontiguousarray(packed).astype(ml_dtypes.bfloat16)


def _pack_v_stack(w):
    """w: [192(d), 192(c), 3, 3] fp32 -> [192(d), 14, 128] bf16: Wv arranged
    by contraction-chunk rows so FW^T chunks come out of one matmul."""
    # wt[d, t, c]
    wt = np.ascontiguousarray(w.transpose(0, 2, 3, 1).reshape(DIM, 9, DIM))
    packed = np.zeros((DIM, 14, 128), dtype=np.float32)
    for j in range(9):
        packed[:, j, :] = wt[:, j, 0:128]
    for j, t in enumerate((0, 2, 4, 6, 8)):
        packed[:, 9 + j, 0:64] = wt[:, t, 128:192]
        if t + 1 < 9:
            packed[:, 9 + j, 64:128] = wt[:, t + 1, 128:192]
    return np.ascontiguousarray(packed).astype(ml_dtypes.bfloat16)


def prep_in_maps(x, qkv_w, proj_w, proj_b, temperature):
    bf16 = ml_dtypes.bfloat16
    n = x.shape[0]
    assert x.shape == (n, DIM, H, W)
    xpad = np.zeros((n, DIM, XF), dtype=bf16)
    tmp = np.zeros((n, DIM, HP, WP), dtype=bf16)
    tmp[:, :, 1 : H + 1, 1 : W + 1] = x.astype(bf16)
    xpad[:, :, : HP * WP] = tmp.reshape(n, DIM, HP * WP)
    wqk = _pack_qk_weights(qkv_w[: 2 * DIM])
    wvs = _pack_v_stack(qkv_w[2 * DIM :])
    wpt = np.ascontiguousarray(proj_w[:, :, 0, 0].T).astype(bf16)
    biasc = np.ascontiguousarray(proj_b.reshape(DIM, 1)).astype(np.float32)
    tq = np.repeat(np.asarray(temperature, np.float32).reshape(HEADS), DIM // HEADS)
    tq = np.ascontiguousarray(tq.reshape(DIM, 1))
    mask = np.kron(np.eye(HEADS, dtype=np.float32),
                   np.ones((DIM // HEADS, DIM // HEADS), np.float32))
    mask = np.ascontiguousarray(mask)
    idn = np.eye(128, dtype=np.float32)
    shared = {"wqk": wqk, "wvs": wvs, "wpt": wpt, "bias": biasc,
              "tempq": tq, "maskblk": mask, "ident": idn}
    return [{"xp": np.ascontiguousarray(xpad[i]), **shared} for i in range(n)]


def kernel(x, qkv_w, proj_w, proj_b, temperature):
    x = np.asarray(x, np.float32)
    qkv_w = np.asarray(qkv_w, np.float32)
    proj_w = np.asarray(proj_w, np.float32)
    proj_b = np.asarray(proj_b, np.float32)
    temperature = np.asarray(temperature, np.float32)
    nc = build_program()
    in_maps = prep_in_maps(x, qkv_w, proj_w, proj_b, temperature)
    res = run_bass_kernel_spmd(nc, in_maps, core_ids=list(range(NCORES)))
    outs = [res.results[i]["out"].reshape(DIM, H, W) for i in range(NCORES)]
    return np.stack(outs, axis=0).astype(np.float32)
